# revision 1
# baseline (speedup 1.0000x reference)
"""Bidirectional cross-attention kernel for Trainium2, SPMD over 8 NeuronCores.

Reference (per batch b, heads K=8, head dim D=32, N=128*128 pixels):
    q   = softmax_d(Wq @ x)
    for branch j in {1,2}:
        key   = softmax_n(Wk_j @ ref_j)          # softmax over the pixel dim
        v     = Wv_j @ ref_j
        ctx_j = key @ v^T                        # [K,D,D]
        out_j = per-pixel  q @ ctx_j^T
    y = Wo @ concat(out_1, out_2)

Sharding: 8 cores = batch(4) x head-group(2).  Each core owns 4 of the 8
heads for its batch: projections, softmaxes, ctx and the out einsum are
fully head-local; the final Wo projection is computed as a partial sum
over the core's 256 (of 512) concat channels, and the host adds the two
partial outputs per batch.  No cross-core communication on device.

Numerics: bf16 matmul inputs (host-cast), fp32 PSUM accumulation, fp32
scalar/vector math.  Softmaxes skip max-subtraction (logits ~N(0,1), exp
is safe in fp32).

SBUF layout: tensors with >128 channels are stored as [128, k*cols] with
128-channel k-tiles side by side in the free dim.  Key/value tensors are
kept in transposed [pixel, channel] layout (needed for the ctx einsum,
whose contraction runs over pixels).
"""

import numpy as np
import ml_dtypes

import concourse.bass as bass
import concourse.bacc as bacc
import concourse.tile as tile
from concourse import mybir
from concourse.bass_utils import run_bass_kernel_spmd

BF16 = mybir.dt.bfloat16
F32 = mybir.dt.float32
AF = mybir.ActivationFunctionType

B, C, H, W = 4, 256, 128, 128
K, D = 8, 32
N = H * W
N_CORES = 8


def build_nc(n_loc=N):
    nc = bacc.Bacc("TRN2", target_bir_lowering=False, debug=False,
                   num_devices=N_CORES)

    nt = n_loc // 128        # 128-pixel tiles (128)
    nt512 = n_loc // 512     # 512-pixel tiles (32)

    # ---- I/O (weights pre-transposed, head-group-sliced, k-tiled on host) --
    x = nc.declare_dram_parameter("x", [C, n_loc], BF16, isOutput=False)
    r1 = nc.declare_dram_parameter("r1", [C, n_loc], BF16, isOutput=False)
    r2 = nc.declare_dram_parameter("r2", [C, n_loc], BF16, isOutput=False)
    # wq: [128, 2*128]  col chunk 128k = Wq.T[128k:128k+128, our 128 channels]
    wq = nc.declare_dram_parameter("wq", [128, 2 * 128], BF16, isOutput=False)
    # wkv_j: [128, 2*256] col chunk 256k = [WkT | WvT](our heads)[128k:, :]
    wkv1 = nc.declare_dram_parameter("wkv1", [128, 2 * 256], BF16, isOutput=False)
    wkv2 = nc.declare_dram_parameter("wkv2", [128, 2 * 256], BF16, isOutput=False)
    # wo: [128, 2*256]  col chunk 256k = Wo.T[our 256 concat channels][128k:, :]
    wo = nc.declare_dram_parameter("wo", [128, 2 * 256], BF16, isOutput=False)
    ones4 = nc.declare_dram_parameter("ones4", [128, 32], BF16, isOutput=False)
    ones4T = nc.declare_dram_parameter("ones4T", [128, 128], BF16, isOutput=False)
    ones1 = nc.declare_dram_parameter("ones1", [128, 1], BF16, isOutput=False)

    y = nc.declare_dram_parameter("y", [C, n_loc], BF16, isOutput=True)

    refs = [r1, r2]

    with tile.TileContext(nc) as tc:
        with (
            tc.tile_pool(name="weights", bufs=1) as wpool,
            tc.tile_pool(name="persist", bufs=1) as ppool,
            tc.tile_pool(name="io", bufs=3) as iopool,
            tc.tile_pool(name="work", bufs=3) as wkpool,
        ):
            # ---- weights / constants ----
            wq_t = wpool.tile([128, 2 * 128], BF16, tag="wq")
            nc.sync.dma_start(wq_t[:], wq[:, :])
            wkv_t = []
            for j, wsrc in enumerate((wkv1, wkv2)):
                t = wpool.tile([128, 2 * 256], BF16, tag=f"wkv{j}", name=f"wkv_t{j}")
                (nc.sync if j == 0 else nc.gpsimd).dma_start(t[:], wsrc[:, :])
                wkv_t.append(t)
            wo_t = wpool.tile([128, 2 * 256], BF16, tag="wo")
            nc.gpsimd.dma_start(wo_t[:], wo[:, :])
            ones4_t = wpool.tile([128, 32], BF16, tag="o4")
            nc.sync.dma_start(ones4_t[:], ones4[:, :])
            ones4T_t = wpool.tile([128, 128], BF16, tag="o4T")
            nc.gpsimd.dma_start(ones4T_t[:], ones4T[:, :])
            ones1_t = wpool.tile([128, 1], BF16, tag="o1")
            nc.gpsimd.dma_start(ones1_t[:], ones1[:, :])

            compact = ppool.tile([128, 64], F32, tag="compact")
            zk_sb = ppool.tile([1, 256], F32, tag="zk_sb")
            expq = ppool.tile([128, n_loc], BF16, tag="expq")
            nzc = (nt512 + 3) // 4
            zqr = ppool.tile([128, 512 * nzc], BF16, tag="zqr")
            zkT_sb = ppool.tile([128, 2], F32, tag="zkT_sb")

            CH = 4               # kv: 128-pixel tiles per chunk
            nch = nt // CH       # 32 chunks per branch

            with (
                tc.tile_pool(name="kvstage", bufs=1) as kvpool,
                tc.tile_pool(name="psA", bufs=2, space="PSUM") as psA,
                tc.tile_pool(name="psAcc", bufs=1, space="PSUM") as psAcc,
                tc.tile_pool(name="psQ", bufs=2, space="PSUM") as psQ,
            ):
                ekt_all = kvpool.tile([128, nt * 128], BF16, tag="ekt_all")
                vt_all = kvpool.tile([128, nt * 128], BF16, tag="vt_all")
                # ctx blob: cols 128j = branch j cross-blocks; 256:258 = zkT
                ctx_ps = psAcc.tile([128, 512], F32, tag="ctx")
                zk_ps = psAcc.tile([1, 256], F32, tag="zk")

                def pass1(j, ch):
                    base = ch * CH * 128
                    r_t = iopool.tile([128, CH * 256], BF16, tag="rchunk",
                                      name=f"r_{j}_{ch}")
                    for k in range(2):
                        dma_eng = nc.sync if (ch + k) % 2 == 0 else nc.gpsimd
                        dma_eng.dma_start(
                            r_t[:, CH * 128 * k:CH * 128 * (k + 1)],
                            refs[j][128 * k:128 * (k + 1), base:base + CH * 128])
                    kv_ps = psA.tile([128, CH * 256], F32, tag="kv",
                                     name=f"kv_{j}_{ch}")
                    for t in range(CH):
                        for k in range(2):
                            nc.tensor.matmul(
                                kv_ps[:, 256 * t:256 * (t + 1)],
                                r_t[:, CH * 128 * k + 128 * t:
                                       CH * 128 * k + 128 * (t + 1)],
                                wkv_t[j][:, 256 * k:256 * (k + 1)],
                                start=(k == 0), stop=(k == 1),
                            )
                    ek_sl = ekt_all[:, ch * CH * 128:(ch + 1) * CH * 128]
                    nc.scalar.activation(
                        ek_sl.rearrange("p (t c) -> p t c", t=CH),
                        kv_ps[:].rearrange("p (t c) -> p t c", t=CH)[:, :, 0:128],
                        AF.Exp,
                    )
                    vt_sl = vt_all[:, ch * CH * 128:(ch + 1) * CH * 128]
                    nc.vector.tensor_copy(
                        vt_sl.rearrange("p (t c) -> p t c", t=CH),
                        kv_ps[:].rearrange("p (t c) -> p t c", t=CH)[:, :, 128:256],
                    )

                def pass2(j, ch):
                    # ctx + zk accumulation for the CH tiles of chunk ch
                    for t in range(ch * CH, (ch + 1) * CH):
                        nc.tensor.matmul(
                            ctx_ps[:, 128 * j:128 * (j + 1)],
                            vt_all[:, 128 * t:128 * (t + 1)],
                            ekt_all[:, 128 * t:128 * (t + 1)],
                            start=(t == 0), stop=(t == nt - 1),
                        )
                    for t in range(ch * CH, (ch + 1) * CH):
                        nc.tensor.matmul(
                            zk_ps[:, 128 * j:128 * (j + 1)],
                            ones1_t[:],
                            ekt_all[:, 128 * t:128 * (t + 1)],
                            start=(t == 0), stop=(t == nt - 1),
                        )

                def compact_j(j):
                    for a in range(4):
                        nc.vector.tensor_copy(
                            compact[32 * a:32 * (a + 1), 32 * j:32 * j + 32],
                            ctx_ps[32 * a:32 * (a + 1),
                                   128 * j + 32 * a:128 * j + 32 * (a + 1)],
                        )

                def qchunk(i):
                    base = i * 512
                    x_t = iopool.tile([128, 1024], BF16, tag="xchunk",
                                      name=f"x_{i}")
                    for k in range(2):
                        dma_eng = nc.sync if (i + k) % 2 == 0 else nc.gpsimd
                        dma_eng.dma_start(
                            x_t[:, 512 * k:512 * (k + 1)],
                            x[128 * k:128 * (k + 1), base:base + 512])
                    q_ps = psQ.tile([128, 512], F32, tag="q", name=f"q_{i}")
                    for k in range(2):
                        nc.tensor.matmul(
                            q_ps[:], wq_t[:, 128 * k:128 * (k + 1)],
                            x_t[:, 512 * k:512 * (k + 1)],
                            start=(k == 0), stop=(k == 1),
                        )
                    nc.scalar.activation(
                        expq[:, base:base + 512], q_ps[:], AF.Exp)

                def zqchunk(tc4):
                    zq_ps = psQ.tile([128, 512], F32, tag="q", name=f"zq_{tc4}")
                    for u in range(4):
                        t = 4 * tc4 + u
                        nc.tensor.matmul(
                            zq_ps[32 * u:32 * u + 32, :], ones4_t[:],
                            expq[:, 512 * t:512 * (t + 1)],
                            start=True, stop=True,
                            tile_position=(0, 32 * u),
                        )
                    zq_f = wkpool.tile([128, 512], F32, tag="zq_f",
                                       name=f"zqf_{tc4}")
                    nc.vector.reciprocal_approx_fast(zq_f[:], zq_ps[:])
                    nc.vector.tensor_copy(
                        zqr[:, 512 * tc4:512 * (tc4 + 1)], zq_f[:])

                # ---- branch 0: kv chunks with trailing ctx/zk batches ----
                for ch in range(nch):
                    pass1(0, ch)
                    if ch > 0:
                        pass2(0, ch - 1)
                pass2(0, nch - 1)
                compact_j(0)

                # ---- branch 1 interleaved with q projection + zq ----
                for ch in range(nch):
                    pass1(1, ch)
                    if ch > 0:
                        pass2(1, ch - 1)
                    qchunk(ch)
                    if ch % 4 == 3:
                        zqchunk(ch // 4)
                        for t in range(4 * (ch // 4), 4 * (ch // 4) + 4):
                            u, tc4 = t % 4, t // 4
                            zqb_ps = psQ.tile([128, 512], F32, tag="q",
                                              name=f"zqb_{t}")
                            nc.tensor.matmul(
                                zqb_ps[:], ones4T_t[32 * u:32 * u + 4, :],
                                zqr[32 * u:32 * u + 4,
                                    512 * tc4:512 * (tc4 + 1)],
                                start=True, stop=True,
                                tile_position=(32 * u, 0),
                            )
                            nc.vector.tensor_mul(
                                expq[:, 512 * t:512 * (t + 1)],
                                expq[:, 512 * t:512 * (t + 1)],
                                zqb_ps[:],
                            )
                pass2(1, nch - 1)
                compact_j(1)

                # ---- zk: transpose onto partitions ----
                onesf = ppool.tile([1, 1], F32, tag="onesf")
                nc.vector.memset(onesf[:], 1.0)
                nc.vector.tensor_copy(zk_sb[:], zk_ps[:])
                zkT_ps = ctx_ps[:, 256:258]
                nc.tensor.matmul(zkT_ps[:, 0:1], zk_sb[0:1, 0:128],
                                 onesf[:], start=True, stop=True)
                # branch-2 column is emitted block-rotated (+32 partitions)
                # to match the rotated einsum col strips of cat k-tile 1.
                for o in range(4):
                    a = (o - 1) % 4
                    nc.tensor.matmul(
                        zkT_ps[32 * o:32 * (o + 1), 1:2],
                        zk_sb[0:1, 128 + 32 * a:128 + 32 * (a + 1)],
                        onesf[:], start=True, stop=True,
                        tile_position=(0, 32 * o))
                nc.vector.tensor_copy(zkT_sb[:], zkT_ps[:])

            recipT = ppool.tile([128, 2], F32, tag="recipT")
            nc.vector.reciprocal_approx_fast(recipT[:], zkT_sb[:])
            ctxT = ppool.tile([128, 64], BF16, tag="ctxT")
            nc.vector.tensor_copy(ctxT[:], compact[:])

            # ======= Phase C: zq-normalize + einsum + Wo, one pipeline ======
            cat_all = ppool.tile([128, 2 * n_loc], BF16, tag="cat_all")
            with tc.tile_pool(name="psC", bufs=2, space="PSUM") as psC:

                def einsum_tile(t):
                    cat_pss = [psC.tile([128, 512], F32, tag="cat_ps",
                                        name=f"cat_{t}_{j}", bufs=4)
                               for j in range(2)]
                    for j in range(2):
                        for a in range(4):
                            o = (a + j) % 4  # j1 uses rotated column strips
                            nc.tensor.matmul(
                                cat_pss[j][32 * o:32 * (o + 1), :],
                                ctxT[32 * a:32 * (a + 1), 32 * j:32 * j + 32],
                                expq[32 * a:32 * (a + 1), 512 * t:512 * (t + 1)],
                                start=True, stop=True,
                                tile_position=(32 * a, 32 * o),
                            )
                    for j in range(2):
                        # rows of cat k-tile j1 are block-rotated; the host
                        # rotates the matching Wo rows to compensate.  recipT
                        # col j must be rotated the same way on host.
                        csl = cat_all[:, n_loc * j + 512 * t:
                                         n_loc * j + 512 * (t + 1)]
                        if (t + j) % 2 == 0:
                            nc.vector.tensor_scalar_mul(
                                csl, cat_pss[j][:], recipT[:, j:j + 1])
                        else:
                            nc.scalar.activation(
                                csl, cat_pss[j][:], AF.Copy,
                                scale=recipT[:, j:j + 1])

                def wo_tile(t):
                    y_sb = wkpool.tile([128, 2 * 512], BF16, tag="ysb",
                                       name=f"ysb_{t}")
                    for m in range(2):
                        y_ps = psC.tile([128, 512], F32, tag="y_ps",
                                        name=f"yps_{t}_{m}", bufs=4)
                        for k in range(2):
                            nc.tensor.matmul(
                                y_ps[:],
                                wo_t[:, 256 * k + 128 * m:256 * k + 128 * (m + 1)],
                                cat_all[:, n_loc * k + 512 * t:
                                           n_loc * k + 512 * (t + 1)],
                                start=(k == 0), stop=(k == 1),
                            )
                        if m == 0:
                            nc.vector.tensor_copy(
                                y_sb[:, 512 * m:512 * (m + 1)], y_ps[:])
                        else:
                            nc.scalar.copy(
                                y_sb[:, 512 * m:512 * (m + 1)], y_ps[:])
                    for m in range(2):
                        dma_eng = nc.sync if (t + m) % 2 == 0 else nc.gpsimd
                        dma_eng.dma_start(
                            y[128 * m:128 * (m + 1), 512 * t:512 * (t + 1)],
                            y_sb[:, 512 * m:512 * (m + 1)])

                LAG = 4
                for t in range(nt512):
                    einsum_tile(t)
                    if t >= LAG:
                        wo_tile(t - LAG)
                for t in range(nt512 - LAG, nt512):
                    wo_tile(t)

    nc.compile()
    return nc


def _consts():
    ones4 = np.zeros((128, 32), dtype=ml_dtypes.bfloat16)
    for col in range(32):
        a = col % 4
        ones4[32 * a:32 * (a + 1), col] = 1
    ones4T = np.zeros((128, 128), dtype=ml_dtypes.bfloat16)
    for u in range(4):
        for a in range(4):
            ones4T[32 * u + a, 32 * a:32 * (a + 1)] = 1
    ones1 = np.ones((128, 1), dtype=ml_dtypes.bfloat16)
    return ones4, ones4T, ones1


def _ktile(wT):
    """[C_in, C_out] -> [128, (C_in//128)*C_out] k-tiles along the free dim."""
    kin = wT.shape[0] // 128
    return np.concatenate([wT[128 * k:128 * (k + 1), :] for k in range(kin)], axis=1)


def make_in_maps(x, ref_1, ref_2, Wq, Wk1, Wk2, Wv1, Wv2, Wo, n_loc=N):
    bf = ml_dtypes.bfloat16
    ones4, ones4T, ones1 = _consts()
    xf = np.asarray(x).reshape(B, C, -1)
    r1f = np.asarray(ref_1).reshape(B, C, -1)
    r2f = np.asarray(ref_2).reshape(B, C, -1)
    WqT, WoT = np.asarray(Wq).T, np.asarray(Wo).T
    WkT = [np.asarray(Wk1).T, np.asarray(Wk2).T]
    WvT = [np.asarray(Wv1).T, np.asarray(Wv2).T]
    gw = {}
    for g in range(2):
        sl = slice(128 * g, 128 * (g + 1))
        wq_g = np.ascontiguousarray(_ktile(WqT[:, sl])).astype(bf)
        wkv_g = [np.ascontiguousarray(
            _ktile(np.concatenate([WkT[j][:, sl], WvT[j][:, sl]], axis=1))
        ).astype(bf) for j in range(2)]
        # Wo rows for our concat channels: branch1 128g.., branch2 256+128g..
        wo_rows = np.concatenate(
            [WoT[sl, :],
             np.roll(WoT[256 + 128 * g:256 + 128 * (g + 1), :], 32, axis=0)],
            axis=0)
        wo_g = np.ascontiguousarray(_ktile(wo_rows)).astype(bf)
        gw[g] = (wq_g, wkv_g[0], wkv_g[1], wo_g)
    in_maps = []
    for core in range(N_CORES):
        b, g = core // 2, core % 2
        wq_g, wkv1_g, wkv2_g, wo_g = gw[g]
        in_maps.append({
            "x": np.ascontiguousarray(xf[b, :, :n_loc]).astype(bf),
            "r1": np.ascontiguousarray(r1f[b, :, :n_loc]).astype(bf),
            "r2": np.ascontiguousarray(r2f[b, :, :n_loc]).astype(bf),
            "wq": wq_g, "wkv1": wkv1_g, "wkv2": wkv2_g, "wo": wo_g,
            "ones4": ones4, "ones4T": ones4T, "ones1": ones1,
        })
    return in_maps


_NC_CACHE = {}


def kernel(x, ref_1, ref_2, Wq, Wk1, Wk2, Wv1, Wv2, Wo, _trace=False):
    n_loc = N
    if n_loc not in _NC_CACHE:
        _NC_CACHE[n_loc] = build_nc(n_loc)
    nc = _NC_CACHE[n_loc]
    in_maps = make_in_maps(x, ref_1, ref_2, Wq, Wk1, Wk2, Wv1, Wv2, Wo, n_loc)
    res = run_bass_kernel_spmd(nc, in_maps, core_ids=list(range(N_CORES)),
                               trace=_trace)
    out = np.empty((B, C, n_loc), dtype=np.float32)
    for b in range(B):
        out[b] = (res.results[2 * b]["y"].astype(np.float32)
                  + res.results[2 * b + 1]["y"].astype(np.float32))
    if _trace:
        kernel.last_results = res
    return out.reshape(B, C, H, W)



# revision 3
# speedup vs baseline: 1.2351x; 1.2351x over previous
"""Bidirectional cross-attention kernel for Trainium2, SPMD over 8 NeuronCores.

Reference (per batch b, heads K=8, head dim D=32, N=128*128 pixels):
    q   = softmax_d(Wq @ x)
    for branch j in {1,2}:
        key   = softmax_n(Wk_j @ ref_j)          # softmax over the pixel dim
        v     = Wv_j @ ref_j
        ctx_j = key @ v^T                        # [K,D,D]
        out_j = per-pixel  q @ ctx_j^T
    y = Wo @ concat(out_1, out_2)

Sharding: 8 cores = batch(4) x head-group(2).  Each core owns 4 of the 8
heads for its batch: projections, softmaxes, ctx and the out einsum are
fully head-local; the final Wo projection is computed as a partial sum
over the core's 256 (of 512) concat channels, and the host adds the two
partial outputs per batch.  No cross-core communication on device.

Key algebraic restructure vs the straightforward version: since the
per-pixel out einsum and Wo are both linear, fold them:
    y = sum_j Wo_j @ (ctx_norm_j^T @ q) = (sum_j Wo_j @ ctx_norm_j) @ q
      = WF @ q
where WF is a per-head-block [256, 32] matrix built once from the tiny
ctx blocks.  This turns the whole output phase into a single
[256x128] @ [128xN] matmul stream and removes the concat buffer and its
per-pixel zk normalization entirely (zk folds into WF).

zk (sum over pixels of exp(k)) is obtained for free by appending a
ones-column to the streamed v operand of the ctx matmul: ctx is computed
as ek^T-stationary x [v | 1], so column D of the ctx PSUM block is zk,
already transposed onto partitions.

Numerics: bf16 matmul inputs (host-cast), fp32 PSUM accumulation, fp32
scalar/vector math.  Softmaxes skip max-subtraction (logits ~N(0,1), exp
is safe in fp32).

SBUF layout: tensors with >128 channels are stored as [128, k*cols] with
128-channel k-tiles side by side in the free dim.  Key/value tensors are
kept in transposed [pixel, channel] layout (needed for the ctx einsum,
whose contraction runs over pixels); v tiles are 129 wide (128 channels
+ a ones column).
"""

import numpy as np
import ml_dtypes

import concourse.bass as bass
import concourse.bacc as bacc
import concourse.tile as tile
from concourse import mybir
from concourse.bass_utils import run_bass_kernel_spmd

BF16 = mybir.dt.bfloat16
F32 = mybir.dt.float32
AF = mybir.ActivationFunctionType

B, C, H, W = 4, 256, 128, 128
K, D = 8, 32
N = H * W
N_CORES = 8


def build_nc(n_loc=N):
    nc = bacc.Bacc("TRN2", target_bir_lowering=False, debug=False,
                   num_devices=N_CORES)

    nt = n_loc // 128        # 128-pixel tiles (128)
    nt512 = n_loc // 512     # 512-pixel tiles (32)

    # ---- I/O (weights pre-transposed, head-group-sliced, k-tiled on host) --
    x = nc.declare_dram_parameter("x", [C, n_loc], BF16, isOutput=False)
    r1 = nc.declare_dram_parameter("r1", [C, n_loc], BF16, isOutput=False)
    r2 = nc.declare_dram_parameter("r2", [C, n_loc], BF16, isOutput=False)
    # wq: [128, 2*128]  col chunk 128k = Wq.T[128k:128k+128, our 128 channels]
    wq = nc.declare_dram_parameter("wq", [128, 2 * 128], BF16, isOutput=False)
    # wkv_j: [128, 2*256] col chunk 256k = [WkT | WvT](our heads)[128k:, :]
    wkv1 = nc.declare_dram_parameter("wkv1", [128, 2 * 256], BF16, isOutput=False)
    wkv2 = nc.declare_dram_parameter("wkv2", [128, 2 * 256], BF16, isOutput=False)
    # wo: [128, 2*256]  col chunk 256j = Wo.T[(branch j, our heads) rows, :]
    wo = nc.declare_dram_parameter("wo", [128, 2 * 256], BF16, isOutput=False)
    ones4 = nc.declare_dram_parameter("ones4", [128, 32], BF16, isOutput=False)
    ones4T = nc.declare_dram_parameter("ones4T", [128, 128], BF16, isOutput=False)

    y = nc.declare_dram_parameter("y", [C, n_loc], BF16, isOutput=True)

    refs = [r1, r2]
    y3 = y.rearrange("(m p) c -> p m c", m=2)
    x3 = x.rearrange("(k p) c -> p k c", k=2)
    r3 = [r.rearrange("(k p) c -> p k c", k=2) for r in refs]

    with tile.TileContext(nc) as tc:
        with (
            tc.tile_pool(name="weights", bufs=1) as wpool,
            tc.tile_pool(name="persist", bufs=1) as ppool,
            tc.tile_pool(name="io", bufs=6) as iopool,
            tc.tile_pool(name="work", bufs=3) as wkpool,
        ):
            # ---- weights / constants (wkv1 first: branch 0 needs it now) ---
            wkv_t = []
            for j, wsrc in enumerate((wkv1, wkv2)):
                t = wpool.tile([128, 2 * 256], BF16, tag=f"wkv{j}", name=f"wkv_t{j}")
                (nc.gpsimd if j == 0 else nc.sync).dma_start(t[:], wsrc[:, :])
                wkv_t.append(t)
            wq_t = wpool.tile([128, 2 * 128], BF16, tag="wq")
            nc.sync.dma_start(wq_t[:], wq[:, :])
            ones4_t = wpool.tile([128, 32], BF16, tag="o4")
            nc.gpsimd.dma_start(ones4_t[:], ones4[:, :])
            ones4T_t = wpool.tile([128, 128], BF16, tag="o4T")
            nc.sync.dma_start(ones4T_t[:], ones4T[:, :])
            wo_t = wpool.tile([128, 2 * 256], BF16, tag="wo")
            nc.gpsimd.dma_start(wo_t[:], wo[:, :])

            expq = ppool.tile([128, n_loc], BF16, tag="expq")
            nzc = (nt512 + 3) // 4
            zqr = ppool.tile([128, 512 * nzc], BF16, tag="zqr")
            recips = ppool.tile([128, 2], F32, tag="recips")
            compact = ppool.tile([128, 64], BF16, tag="compact")
            wft_sb = ppool.tile([128, 256], BF16, tag="wft_sb")

            CH = 4               # kv: 128-pixel tiles per chunk
            nch = nt // CH       # 32 chunks per branch

            with (
                tc.tile_pool(name="kvstage", bufs=1) as kvpool,
                tc.tile_pool(name="psA", bufs=2, space="PSUM") as psA,
                tc.tile_pool(name="psAcc", bufs=1, space="PSUM") as psAcc,
                tc.tile_pool(name="psQ", bufs=2, space="PSUM") as psQ,
            ):
                ekt_all = kvpool.tile([128, nt * 128], BF16, tag="ekt_all")
                vt_all = kvpool.tile([128, nt * 129], BF16, tag="vt_all")
                vt_v = vt_all.rearrange("p (t c) -> p t c", c=129)
                # ones column per v tile (survives both branches: the per-
                # chunk v copies only touch cols 0:128 of each 129-block)
                nc.vector.memset(vt_v[:, :, 128:129], 1.0)
                # ctx blob: branch j at cols 129j..129j+129; col 129j+128 = zk
                ctx_ps = psAcc.tile([128, 2 * 129], F32, tag="ctx")

                def pass1(j, ch):
                    base = ch * CH * 128
                    r_t = iopool.tile([128, CH * 256], BF16, tag="rchunk",
                                      name=f"r_{j}_{ch}")
                    dma_eng = nc.sync if ch % 2 == 0 else nc.gpsimd
                    dma_eng.dma_start(
                        r_t.rearrange("p (k c) -> p k c", k=2),
                        r3[j][:, :, base:base + CH * 128])
                    kv_ps = psA.tile([128, CH * 256], F32, tag="kv",
                                     name=f"kv_{j}_{ch}")
                    for t in range(CH):
                        for k in range(2):
                            nc.tensor.matmul(
                                kv_ps[:, 256 * t:256 * (t + 1)],
                                r_t[:, CH * 128 * k + 128 * t:
                                       CH * 128 * k + 128 * (t + 1)],
                                wkv_t[j][:, 256 * k:256 * (k + 1)],
                                start=(k == 0), stop=(k == 1),
                            )
                    ek_sl = ekt_all[:, ch * CH * 128:(ch + 1) * CH * 128]
                    nc.scalar.activation(
                        ek_sl.rearrange("p (t c) -> p t c", t=CH),
                        kv_ps[:].rearrange("p (t c) -> p t c", t=CH)[:, :, 0:128],
                        AF.Exp,
                    )
                    nc.vector.tensor_copy(
                        vt_v[:, ch * CH:(ch + 1) * CH, 0:128],
                        kv_ps[:].rearrange("p (t c) -> p t c", t=CH)[:, :, 128:256],
                    )

                def pass2(j, ch):
                    # ctx accumulation: ek-tile stationary, [v | 1] streamed.
                    # out[c, d] = sum_pix ek[pix, c] v[pix, d]; col 128 = zk.
                    for t in range(ch * CH, (ch + 1) * CH):
                        nc.tensor.matmul(
                            ctx_ps[:, 129 * j:129 * (j + 1)],
                            ekt_all[:, 128 * t:128 * (t + 1)],
                            vt_all[:, 129 * t:129 * (t + 1)],
                            start=(t == 0), stop=(t == nt - 1),
                        )

                def qchunk(i):
                    base = i * 512
                    x_t = iopool.tile([128, 1024], BF16, tag="xchunk",
                                      name=f"x_{i}")
                    dma_eng = nc.gpsimd if i % 2 == 0 else nc.sync
                    dma_eng.dma_start(
                        x_t.rearrange("p (k c) -> p k c", k=2),
                        x3[:, :, base:base + 512])
                    q_ps = psQ.tile([128, 512], F32, tag="q", name=f"q_{i}")
                    for k in range(2):
                        nc.tensor.matmul(
                            q_ps[:], wq_t[:, 128 * k:128 * (k + 1)],
                            x_t[:, 512 * k:512 * (k + 1)],
                            start=(k == 0), stop=(k == 1),
                        )
                    nc.scalar.activation(
                        expq[:, base:base + 512], q_ps[:], AF.Exp)

                def zqgroup(tc4):
                    # zq = per-head sums of expq (4 col-tiled concurrent MMs),
                    # reciprocal, then matmul-broadcast back over the 32
                    # partitions of each head and normalize expq in place.
                    zq_ps = psQ.tile([128, 512], F32, tag="q", name=f"zq_{tc4}")
                    for u in range(4):
                        t = 4 * tc4 + u
                        nc.tensor.matmul(
                            zq_ps[32 * u:32 * u + 32, :], ones4_t[:],
                            expq[:, 512 * t:512 * (t + 1)],
                            start=True, stop=True,
                            tile_position=(0, 32 * u),
                        )
                    zq_f = wkpool.tile([128, 512], F32, tag="zq_f",
                                       name=f"zqf_{tc4}")
                    nc.vector.reciprocal_approx_fast(zq_f[:], zq_ps[:])
                    nc.vector.tensor_copy(
                        zqr[:, 512 * tc4:512 * (tc4 + 1)], zq_f[:])
                    for t in range(4 * tc4, 4 * tc4 + 4):
                        u = t % 4
                        zqb_ps = psQ.tile([128, 512], F32, tag="q",
                                          name=f"zqb_{t}")
                        nc.tensor.matmul(
                            zqb_ps[:], ones4T_t[32 * u:32 * u + 4, :],
                            zqr[32 * u:32 * u + 4,
                                512 * tc4:512 * (tc4 + 1)],
                            start=True, stop=True,
                            tile_position=(32 * u, 0),
                        )
                        nc.vector.tensor_mul(
                            expq[:, 512 * t:512 * (t + 1)],
                            expq[:, 512 * t:512 * (t + 1)],
                            zqb_ps[:],
                        )

                # ---- branches; q projection spread across both to even out
                # the DMA load (r + x/2 per branch) ----
                for j in range(2):
                    for ch in range(nch):
                        pass1(j, ch)
                        if ch > 0:
                            pass2(j, ch - 1)
                        if ch % 2 == 1:
                            qchunk(16 * j + ch // 2)
                        if ch % 8 == 7:
                            zqgroup(4 * j + ch // 8)
                    pass2(j, nch - 1)

                # ---- WF = sum_j WoT_j^T-blocks @ ctx_norm_j, per head -----
                for j in range(2):
                    nc.vector.reciprocal_approx_fast(
                        recips[:, j:j + 1],
                        ctx_ps[:, 129 * j + 128:129 * j + 129])
                for j in range(2):
                    for h in range(4):
                        nc.vector.tensor_scalar_mul(
                            compact[32 * h:32 * (h + 1), 32 * j:32 * j + 32],
                            ctx_ps[32 * h:32 * (h + 1),
                                   129 * j + 32 * h:129 * j + 32 * (h + 1)],
                            recips[32 * h:32 * (h + 1), j:j + 1],
                        )
                wft_ps = psA.tile([128, 256], F32, tag="wft", bufs=1)
                for h in range(4):
                    for j in range(2):
                        nc.tensor.matmul(
                            wft_ps[32 * h:32 * (h + 1), :],
                            compact[32 * h:32 * (h + 1), 32 * j:32 * j + 32],
                            wo_t[32 * h:32 * (h + 1), 256 * j:256 * (j + 1)],
                            start=(j == 0), stop=(j == 1),
                            tile_position=(32 * h, 32 * h),
                        )
                nc.vector.tensor_copy(wft_sb[:], wft_ps[:])

            # ======= Phase C: y = WF @ expq, streamed over pixel tiles ======
            with tc.tile_pool(name="psC", bufs=3, space="PSUM") as psC:
                for t in range(nt512):
                    y_ps = psC.tile([128, 1024], F32, tag="y", name=f"y_{t}")
                    for m in range(2):
                        nc.tensor.matmul(
                            y_ps[:, 512 * m:512 * (m + 1)],
                            wft_sb[:, 128 * m:128 * (m + 1)],
                            expq[:, 512 * t:512 * (t + 1)],
                            start=True, stop=True,
                        )
                    y_sb = wkpool.tile([128, 1024], BF16, tag="ysb",
                                       name=f"ysb_{t}")
                    if t % 2 == 0:
                        nc.vector.tensor_copy(y_sb[:], y_ps[:])
                    else:
                        nc.scalar.copy(y_sb[:], y_ps[:])
                    dma_eng = nc.sync if t % 2 == 0 else nc.gpsimd
                    dma_eng.dma_start(
                        y3[:, :, 512 * t:512 * (t + 1)],
                        y_sb.rearrange("p (m c) -> p m c", m=2))

    nc.compile()
    return nc


def _consts():
    ones4 = np.zeros((128, 32), dtype=ml_dtypes.bfloat16)
    for col in range(32):
        a = col % 4
        ones4[32 * a:32 * (a + 1), col] = 1
    ones4T = np.zeros((128, 128), dtype=ml_dtypes.bfloat16)
    for u in range(4):
        for a in range(4):
            ones4T[32 * u + a, 32 * a:32 * (a + 1)] = 1
    return ones4, ones4T


def _ktile(wT):
    """[C_in, C_out] -> [128, (C_in//128)*C_out] k-tiles along the free dim."""
    kin = wT.shape[0] // 128
    return np.concatenate([wT[128 * k:128 * (k + 1), :] for k in range(kin)], axis=1)


def make_in_maps(x, ref_1, ref_2, Wq, Wk1, Wk2, Wv1, Wv2, Wo, n_loc=N):
    bf = ml_dtypes.bfloat16
    ones4, ones4T = _consts()
    xf = np.asarray(x).reshape(B, C, -1)
    r1f = np.asarray(ref_1).reshape(B, C, -1)
    r2f = np.asarray(ref_2).reshape(B, C, -1)
    WqT, WoT = np.asarray(Wq).T, np.asarray(Wo).T
    WkT = [np.asarray(Wk1).T, np.asarray(Wk2).T]
    WvT = [np.asarray(Wv1).T, np.asarray(Wv2).T]
    gw = {}
    for g in range(2):
        sl = slice(128 * g, 128 * (g + 1))
        wq_g = np.ascontiguousarray(_ktile(WqT[:, sl])).astype(bf)
        wkv_g = [np.ascontiguousarray(
            _ktile(np.concatenate([WkT[j][:, sl], WvT[j][:, sl]], axis=1))
        ).astype(bf) for j in range(2)]
        # Wo rows for our concat channels: branch j block at cols 256j
        wo_g = np.ascontiguousarray(np.concatenate(
            [WoT[128 * g:128 * (g + 1), :],
             WoT[256 + 128 * g:256 + 128 * (g + 1), :]],
            axis=1)).astype(bf)
        gw[g] = (wq_g, wkv_g[0], wkv_g[1], wo_g)
    in_maps = []
    for core in range(N_CORES):
        b, g = core // 2, core % 2
        wq_g, wkv1_g, wkv2_g, wo_g = gw[g]
        in_maps.append({
            "x": np.ascontiguousarray(xf[b, :, :n_loc]).astype(bf),
            "r1": np.ascontiguousarray(r1f[b, :, :n_loc]).astype(bf),
            "r2": np.ascontiguousarray(r2f[b, :, :n_loc]).astype(bf),
            "wq": wq_g, "wkv1": wkv1_g, "wkv2": wkv2_g, "wo": wo_g,
            "ones4": ones4, "ones4T": ones4T,
        })
    return in_maps


_NC_CACHE = {}


def kernel(x, ref_1, ref_2, Wq, Wk1, Wk2, Wv1, Wv2, Wo, _trace=False):
    n_loc = N
    if n_loc not in _NC_CACHE:
        _NC_CACHE[n_loc] = build_nc(n_loc)
    nc = _NC_CACHE[n_loc]
    in_maps = make_in_maps(x, ref_1, ref_2, Wq, Wk1, Wk2, Wv1, Wv2, Wo, n_loc)
    res = run_bass_kernel_spmd(nc, in_maps, core_ids=list(range(N_CORES)),
                               trace=_trace)
    out = np.empty((B, C, n_loc), dtype=np.float32)
    for b in range(B):
        out[b] = (res.results[2 * b]["y"].astype(np.float32)
                  + res.results[2 * b + 1]["y"].astype(np.float32))
    if _trace:
        kernel.last_results = res
    return out.reshape(B, C, H, W)


# revision 13
# speedup vs baseline: 1.3318x; 1.0783x over previous
"""Bidirectional cross-attention kernel for Trainium2, SPMD over 8 NeuronCores.

Reference (per batch b, heads K=8, head dim D=32, N=128*128 pixels):
    q   = softmax_d(Wq @ x)
    for branch j in {1,2}:
        key   = softmax_n(Wk_j @ ref_j)          # softmax over the pixel dim
        v     = Wv_j @ ref_j
        ctx_j = key @ v^T                        # [K,D,D]
        out_j = per-pixel  q @ ctx_j^T
    y = Wo @ concat(out_1, out_2)

Sharding: 8 cores = batch(4) x head-group(2).  Each core owns 4 of the 8
heads for its batch: projections, softmaxes, ctx and the out einsum are
fully head-local; the final Wo projection is computed as a partial sum
over the core's 256 (of 512) concat channels, and the host adds the two
partial outputs per batch.  No cross-core communication on device.

Key algebraic restructure vs the straightforward version: since the
per-pixel out einsum and Wo are both linear, fold them:
    y = sum_j Wo_j @ (ctx_norm_j^T @ q) = (sum_j Wo_j @ ctx_norm_j) @ q
      = WF @ q
where WF is a per-head-block [256, 32] matrix built once from the tiny
ctx blocks.  This turns the whole output phase into a single
[256x128] @ [128xN] matmul stream and removes the concat buffer and its
per-pixel zk normalization entirely (zk folds into WF).

zk (sum over pixels of exp(k)) is obtained for free by appending a
ones-column to the streamed v operand of the ctx matmul: ctx is computed
as ek^T-stationary x [v | 1], so column D of the ctx PSUM block is zk,
already transposed onto partitions.

Numerics: bf16 matmul inputs (host-cast), fp32 PSUM accumulation, fp32
scalar/vector math.  Softmaxes skip max-subtraction (logits ~N(0,1), exp
is safe in fp32).

SBUF layout: tensors with >128 channels are stored as [128, k*cols] with
128-channel k-tiles side by side in the free dim.  Key/value tensors are
kept in transposed [pixel, channel] layout (needed for the ctx einsum,
whose contraction runs over pixels); v tiles are 129 wide (128 channels
+ a ones column).
"""

import numpy as np
import ml_dtypes

import concourse.bass as bass
import concourse.bacc as bacc
import concourse.tile as tile
from concourse import mybir
from concourse.bass_utils import run_bass_kernel_spmd

BF16 = mybir.dt.bfloat16
F32 = mybir.dt.float32
AF = mybir.ActivationFunctionType

B, C, H, W = 4, 256, 128, 128
K, D = 8, 32
N = H * W
N_CORES = 8


def build_nc(n_loc=N):
    nc = bacc.Bacc("TRN2", target_bir_lowering=False, debug=False,
                   num_devices=N_CORES)

    nt = n_loc // 128        # 128-pixel tiles (128)
    nt512 = n_loc // 512     # 512-pixel tiles (32)

    # ---- I/O (weights pre-transposed, head-group-sliced, k-tiled on host) --
    # x/r/y are chunk-major [nchunk*128, 1024]: chunk ch rows 128ch..128ch+128
    # hold pixels 512ch..512ch+512 for both 128-channel k-tiles side by side
    # (col = 512k + c).  Each DMA is then one fully contiguous [128, 1024]
    # block -> 2 KiB packets instead of 1 KiB row fragments.
    x = nc.declare_dram_parameter("x", [(n_loc // 512) * 128, 1024], BF16,
                                  isOutput=False)
    r1 = nc.declare_dram_parameter("r1", [(n_loc // 512) * 128, 1024], BF16,
                                   isOutput=False)
    r2 = nc.declare_dram_parameter("r2", [(n_loc // 512) * 128, 1024], BF16,
                                   isOutput=False)
    # wq: [128, 2*128]  col chunk 128k = Wq.T[128k:128k+128, our 128 channels]
    wq = nc.declare_dram_parameter("wq", [128, 2 * 128], BF16, isOutput=False)
    # wkv_j: [128, 2*256] col chunk 256k = [WkT | WvT](our heads)[128k:, :]
    wkv1 = nc.declare_dram_parameter("wkv1", [128, 2 * 256], BF16, isOutput=False)
    wkv2 = nc.declare_dram_parameter("wkv2", [128, 2 * 256], BF16, isOutput=False)
    # wo: [128, 2*256]  col chunk 256j = Wo.T[(branch j, our heads) rows, :]
    wo = nc.declare_dram_parameter("wo", [128, 2 * 256], BF16, isOutput=False)
    ones4 = nc.declare_dram_parameter("ones4", [128, 32], BF16, isOutput=False)
    ones4T = nc.declare_dram_parameter("ones4T", [128, 128], BF16, isOutput=False)

    y = nc.declare_dram_parameter("y", [(n_loc // 512) * 128, 1024], BF16,
                                  isOutput=True)

    refs = [r1, r2]

    with tile.TileContext(nc) as tc:
        with (
            tc.tile_pool(name="weights", bufs=1) as wpool,
            tc.tile_pool(name="persist", bufs=1) as ppool,
            tc.tile_pool(name="io", bufs=6) as iopool,
            tc.tile_pool(name="work", bufs=3) as wkpool,
        ):
            # ---- weights / constants.  Only wkv1 + ones4 are loaded up
            # front; the rest are issued between the first chunk DMAs so the
            # input stream starts as early as possible.
            wkv_t = [wpool.tile([128, 2 * 256], BF16, tag=f"wkv{j}",
                                name=f"wkv_t{j}") for j in range(2)]
            nc.gpsimd.dma_start(wkv_t[0][:], wkv1[:, :])
            ones4_t = wpool.tile([128, 32], BF16, tag="o4")
            nc.sync.dma_start(ones4_t[:], ones4[:, :])
            wq_t = wpool.tile([128, 2 * 128], BF16, tag="wq")
            ones4T_t = wpool.tile([128, 128], BF16, tag="o4T")
            wo_t = wpool.tile([128, 2 * 256], BF16, tag="wo")

            expq = ppool.tile([128, n_loc], BF16, tag="expq")
            nzc = (nt512 + 3) // 4
            zqr = ppool.tile([128, 512 * nzc], BF16, tag="zqr")
            recips = ppool.tile([128, 2], F32, tag="recips")
            compact = ppool.tile([128, 64], BF16, tag="compact")
            wft_sb = ppool.tile([128, 256], BF16, tag="wft_sb")

            CH = 4               # kv: 128-pixel tiles per chunk
            nch = nt // CH       # 32 chunks per branch

            with (
                tc.tile_pool(name="kvstage", bufs=1) as kvpool,
                tc.tile_pool(name="psA", bufs=2, space="PSUM") as psA,
                tc.tile_pool(name="psAcc", bufs=1, space="PSUM") as psAcc,
                tc.tile_pool(name="psQ", bufs=2, space="PSUM") as psQ,
            ):
                ekt_all = kvpool.tile([128, nt * 128], BF16, tag="ekt_all")
                vt_all = kvpool.tile([128, nt * 129], BF16, tag="vt_all")
                vt_v = vt_all.rearrange("p (t c) -> p t c", c=129)
                # ones column per v tile (survives both branches: the per-
                # chunk v copies only touch cols 0:128 of each 129-block)
                nc.vector.memset(vt_v[:, :, 128:129], 1.0)
                # ctx blob: branch j at cols 129j..129j+129; col 129j+128 = zk
                ctx_ps = psAcc.tile([128, 2 * 129], F32, tag="ctx")

                def pass1(j, ch):
                    r_t = iopool.tile([128, CH * 256], BF16, tag="rchunk",
                                      name=f"r_{j}_{ch}")
                    dma_eng = nc.sync if ch % 2 == 0 else nc.gpsimd
                    dma_eng.dma_start(
                        r_t[:], refs[j][128 * ch:128 * (ch + 1), :])
                    kv_ps = psA.tile([128, CH * 256], F32, tag="kv",
                                     name=f"kv_{j}_{ch}")
                    for t in range(CH):
                        for k in range(2):
                            nc.tensor.matmul(
                                kv_ps[:, 256 * t:256 * (t + 1)],
                                r_t[:, CH * 128 * k + 128 * t:
                                       CH * 128 * k + 128 * (t + 1)],
                                wkv_t[j][:, 256 * k:256 * (k + 1)],
                                start=(k == 0), stop=(k == 1),
                            )
                    ek_sl = ekt_all[:, ch * CH * 128:(ch + 1) * CH * 128]
                    nc.scalar.activation(
                        ek_sl.rearrange("p (t c) -> p t c", t=CH),
                        kv_ps[:].rearrange("p (t c) -> p t c", t=CH)[:, :, 0:128],
                        AF.Exp,
                    )
                    nc.vector.tensor_copy(
                        vt_v[:, ch * CH:(ch + 1) * CH, 0:128],
                        kv_ps[:].rearrange("p (t c) -> p t c", t=CH)[:, :, 128:256],
                    )

                def pass2(j, ch):
                    # ctx accumulation: ek-tile stationary, [v | 1] streamed.
                    # out[c, d] = sum_pix ek[pix, c] v[pix, d]; col 128 = zk.
                    for t in range(ch * CH, (ch + 1) * CH):
                        nc.tensor.matmul(
                            ctx_ps[:, 129 * j:129 * (j + 1)],
                            ekt_all[:, 128 * t:128 * (t + 1)],
                            vt_all[:, 129 * t:129 * (t + 1)],
                            start=(t == 0), stop=(t == nt - 1),
                        )

                def qchunk(i):
                    base = i * 512
                    x_t = iopool.tile([128, 1024], BF16, tag="xchunk",
                                      name=f"x_{i}")
                    dma_eng = nc.gpsimd if i % 2 == 0 else nc.sync
                    dma_eng.dma_start(x_t[:], x[128 * i:128 * (i + 1), :])
                    q_ps = psQ.tile([128, 512], F32, tag="q", name=f"q_{i}")
                    for k in range(2):
                        nc.tensor.matmul(
                            q_ps[:], wq_t[:, 128 * k:128 * (k + 1)],
                            x_t[:, 512 * k:512 * (k + 1)],
                            start=(k == 0), stop=(k == 1),
                        )
                    nc.scalar.activation(
                        expq[:, base:base + 512], q_ps[:], AF.Exp)

                def zqgroup(tc4):
                    # zq = per-head sums of expq (4 col-tiled concurrent MMs),
                    # reciprocal, then matmul-broadcast back over the 32
                    # partitions of each head and normalize expq in place.
                    zq_ps = psQ.tile([128, 512], F32, tag="q", name=f"zq_{tc4}")
                    for u in range(4):
                        t = 4 * tc4 + u
                        nc.tensor.matmul(
                            zq_ps[32 * u:32 * u + 32, :], ones4_t[:],
                            expq[:, 512 * t:512 * (t + 1)],
                            start=True, stop=True,
                            tile_position=(0, 32 * u),
                        )
                    zq_f = wkpool.tile([128, 512], F32, tag="zq_f",
                                       name=f"zqf_{tc4}")
                    nc.vector.reciprocal_approx_fast(zq_f[:], zq_ps[:])
                    nc.vector.tensor_copy(
                        zqr[:, 512 * tc4:512 * (tc4 + 1)], zq_f[:])
                    for t in range(4 * tc4, 4 * tc4 + 4):
                        u = t % 4
                        zqb_ps = psQ.tile([128, 512], F32, tag="q",
                                          name=f"zqb_{t}")
                        nc.tensor.matmul(
                            zqb_ps[:], ones4T_t[32 * u:32 * u + 4, :],
                            zqr[32 * u:32 * u + 4,
                                512 * tc4:512 * (tc4 + 1)],
                            start=True, stop=True,
                            tile_position=(32 * u, 0),
                        )
                        nc.vector.tensor_mul(
                            expq[:, 512 * t:512 * (t + 1)],
                            expq[:, 512 * t:512 * (t + 1)],
                            zqb_ps[:],
                        )

                # ---- branches; q projection spread across both to even out
                # the DMA load (r + x/2 per branch).  Remaining weight DMAs
                # are dripped in between the chunk DMAs. ----
                for j in range(2):
                    for ch in range(nch):
                        pass1(j, ch)
                        if j == 0 and ch == 0:
                            nc.gpsimd.dma_start(wq_t[:], wq[:, :])
                        if j == 0 and ch == 3:
                            nc.sync.dma_start(ones4T_t[:], ones4T[:, :])
                        if j == 0 and ch == 6:
                            nc.sync.dma_start(wkv_t[1][:], wkv2[:, :])
                        if j == 1 and ch == 2:
                            nc.gpsimd.dma_start(wo_t[:], wo[:, :])
                        if ch > 0:
                            pass2(j, ch - 1)
                        if ch % 2 == 1:
                            qchunk(16 * j + ch // 2)
                        if ch % 8 == 7:
                            zqgroup(4 * j + ch // 8)
                    pass2(j, nch - 1)

                # ---- WF = sum_j WoT_j^T-blocks @ ctx_norm_j, per head -----
                for j in range(2):
                    nc.vector.reciprocal_approx_fast(
                        recips[:, j:j + 1],
                        ctx_ps[:, 129 * j + 128:129 * j + 129])
                for j in range(2):
                    for h in range(4):
                        nc.vector.tensor_scalar_mul(
                            compact[32 * h:32 * (h + 1), 32 * j:32 * j + 32],
                            ctx_ps[32 * h:32 * (h + 1),
                                   129 * j + 32 * h:129 * j + 32 * (h + 1)],
                            recips[32 * h:32 * (h + 1), j:j + 1],
                        )
                wft_ps = psA.tile([128, 256], F32, tag="wft", bufs=1)
                for h in range(4):
                    for j in range(2):
                        nc.tensor.matmul(
                            wft_ps[32 * h:32 * (h + 1), :],
                            compact[32 * h:32 * (h + 1), 32 * j:32 * j + 32],
                            wo_t[32 * h:32 * (h + 1), 256 * j:256 * (j + 1)],
                            start=(j == 0), stop=(j == 1),
                            tile_position=(32 * h, 32 * h),
                        )
                nc.vector.tensor_copy(wft_sb[:], wft_ps[:])

            # ======= Phase C: y = WF @ expq, streamed over pixel tiles ======
            with (
                tc.tile_pool(name="psC", bufs=3, space="PSUM") as psC,
                tc.tile_pool(name="ysb", bufs=5) as ysbpool,
            ):
                for t in range(nt512):
                    y_ps = psC.tile([128, 1024], F32, tag="y", name=f"y_{t}")
                    for m in range(2):
                        nc.tensor.matmul(
                            y_ps[:, 512 * m:512 * (m + 1)],
                            wft_sb[:, 128 * m:128 * (m + 1)],
                            expq[:, 512 * t:512 * (t + 1)],
                            start=True, stop=True,
                        )
                    y_sb = ysbpool.tile([128, 1024], BF16, tag="ysb",
                                        name=f"ysb_{t}")
                    if t % 2 == 0:
                        nc.vector.tensor_copy(y_sb[:], y_ps[:])
                    else:
                        nc.scalar.copy(y_sb[:], y_ps[:])
                    dma_eng = nc.sync if t % 2 == 0 else nc.gpsimd
                    dma_eng.dma_start(y[128 * t:128 * (t + 1), :], y_sb[:])

    nc.compile()
    return nc


def _consts():
    ones4 = np.zeros((128, 32), dtype=ml_dtypes.bfloat16)
    for col in range(32):
        a = col % 4
        ones4[32 * a:32 * (a + 1), col] = 1
    ones4T = np.zeros((128, 128), dtype=ml_dtypes.bfloat16)
    for u in range(4):
        for a in range(4):
            ones4T[32 * u + a, 32 * a:32 * (a + 1)] = 1
    return ones4, ones4T


def _ktile(wT):
    """[C_in, C_out] -> [128, (C_in//128)*C_out] k-tiles along the free dim."""
    kin = wT.shape[0] // 128
    return np.concatenate([wT[128 * k:128 * (k + 1), :] for k in range(kin)], axis=1)


def _chunkmajor(arr, n_loc=N):
    """[256, n] -> [nch*128, 1024]: chunk ch holds pixels 512ch..512ch+512
    with the two 128-channel k-tiles side by side per partition row."""
    nch = n_loc // 512
    a = arr.reshape(2, 128, nch, 512).transpose(2, 1, 0, 3)   # [ch, p, k, c]
    return a.reshape(nch * 128, 1024)


def _unchunk_y(yarr, n_loc=N):
    """[nt512*128, 1024] -> [256, n] (inverse of the y chunk-major layout:
    per tile t, col 512m + c of partition p is y[128m + p, 512t + c])."""
    nt512 = n_loc // 512
    a = yarr.reshape(nt512, 128, 2, 512).transpose(2, 1, 0, 3)  # [m, p, t, c]
    return a.reshape(C, n_loc)


def make_in_maps(x, ref_1, ref_2, Wq, Wk1, Wk2, Wv1, Wv2, Wo, n_loc=N):
    bf = ml_dtypes.bfloat16
    ones4, ones4T = _consts()
    xf = np.asarray(x).reshape(B, C, -1)
    r1f = np.asarray(ref_1).reshape(B, C, -1)
    r2f = np.asarray(ref_2).reshape(B, C, -1)
    WqT, WoT = np.asarray(Wq).T, np.asarray(Wo).T
    WkT = [np.asarray(Wk1).T, np.asarray(Wk2).T]
    WvT = [np.asarray(Wv1).T, np.asarray(Wv2).T]
    gw = {}
    for g in range(2):
        sl = slice(128 * g, 128 * (g + 1))
        wq_g = np.ascontiguousarray(_ktile(WqT[:, sl])).astype(bf)
        wkv_g = [np.ascontiguousarray(
            _ktile(np.concatenate([WkT[j][:, sl], WvT[j][:, sl]], axis=1))
        ).astype(bf) for j in range(2)]
        # Wo rows for our concat channels: branch j block at cols 256j
        wo_g = np.ascontiguousarray(np.concatenate(
            [WoT[128 * g:128 * (g + 1), :],
             WoT[256 + 128 * g:256 + 128 * (g + 1), :]],
            axis=1)).astype(bf)
        gw[g] = (wq_g, wkv_g[0], wkv_g[1], wo_g)
    in_maps = []
    for core in range(N_CORES):
        b, g = core // 2, core % 2
        wq_g, wkv1_g, wkv2_g, wo_g = gw[g]
        in_maps.append({
            "x": np.ascontiguousarray(_chunkmajor(xf[b, :, :n_loc], n_loc)).astype(bf),
            "r1": np.ascontiguousarray(_chunkmajor(r1f[b, :, :n_loc], n_loc)).astype(bf),
            "r2": np.ascontiguousarray(_chunkmajor(r2f[b, :, :n_loc], n_loc)).astype(bf),
            "wq": wq_g, "wkv1": wkv1_g, "wkv2": wkv2_g, "wo": wo_g,
            "ones4": ones4, "ones4T": ones4T,
        })
    return in_maps


_NC_CACHE = {}


def kernel(x, ref_1, ref_2, Wq, Wk1, Wk2, Wv1, Wv2, Wo, _trace=False):
    n_loc = N
    if n_loc not in _NC_CACHE:
        _NC_CACHE[n_loc] = build_nc(n_loc)
    nc = _NC_CACHE[n_loc]
    in_maps = make_in_maps(x, ref_1, ref_2, Wq, Wk1, Wk2, Wv1, Wv2, Wo, n_loc)
    res = run_bass_kernel_spmd(nc, in_maps, core_ids=list(range(N_CORES)),
                               trace=_trace)
    out = np.empty((B, C, n_loc), dtype=np.float32)
    for b in range(B):
        out[b] = (_unchunk_y(res.results[2 * b]["y"].astype(np.float32), n_loc)
                  + _unchunk_y(res.results[2 * b + 1]["y"].astype(np.float32), n_loc))
    if _trace:
        kernel.last_results = res
    return out.reshape(B, C, H, W)


# revision 19
# speedup vs baseline: 1.3508x; 1.0143x over previous
"""Bidirectional cross-attention kernel for Trainium2, SPMD over 8 NeuronCores.

Reference (per batch b, heads K=8, head dim D=32, N=128*128 pixels):
    q   = softmax_d(Wq @ x)
    for branch j in {1,2}:
        key   = softmax_n(Wk_j @ ref_j)          # softmax over the pixel dim
        v     = Wv_j @ ref_j
        ctx_j = key @ v^T                        # [K,D,D]
        out_j = per-pixel  q @ ctx_j^T
    y = Wo @ concat(out_1, out_2)

Sharding: 8 cores = batch(4) x head-group(2).  Each core owns 4 of the 8
heads for its batch: projections, softmaxes, ctx and the out einsum are
fully head-local; the final Wo projection is computed as a partial sum
over the core's 256 (of 512) concat channels, and the host adds the two
partial outputs per batch.  No cross-core communication on device.

Key algebraic restructure vs the straightforward version: since the
per-pixel out einsum and Wo are both linear, fold them:
    y = sum_j Wo_j @ (ctx_norm_j^T @ q) = (sum_j Wo_j @ ctx_norm_j) @ q
      = WF @ q
where WF is a per-head-block [256, 32] matrix built once from the tiny
ctx blocks.  This turns the whole output phase into a single
[256x128] @ [128xN] matmul stream and removes the concat buffer and its
per-pixel zk normalization entirely (zk folds into WF).

zk (sum over pixels of exp(k)) is obtained for free by appending a
ones-column to the streamed v operand of the ctx matmul: ctx is computed
as ek^T-stationary x [v | 1], so column D of the ctx PSUM block is zk,
already transposed onto partitions.

Numerics: bf16 matmul inputs (host-cast), fp32 PSUM accumulation, fp32
scalar/vector math.  Softmaxes skip max-subtraction (logits ~N(0,1), exp
is safe in fp32).

SBUF layout: tensors with >128 channels are stored as [128, k*cols] with
128-channel k-tiles side by side in the free dim.  Key/value tensors are
kept in transposed [pixel, channel] layout (needed for the ctx einsum,
whose contraction runs over pixels); v tiles are 129 wide (128 channels
+ a ones column).
"""

import numpy as np
import ml_dtypes

import concourse.bass as bass
import concourse.bacc as bacc
import concourse.tile as tile
from concourse import mybir
from concourse.bass_utils import run_bass_kernel_spmd

BF16 = mybir.dt.bfloat16
F32 = mybir.dt.float32
AF = mybir.ActivationFunctionType

B, C, H, W = 4, 256, 128, 128
K, D = 8, 32
N = H * W
N_CORES = 8


def build_nc(n_loc=N):
    nc = bacc.Bacc("TRN2", target_bir_lowering=False, debug=False,
                   num_devices=N_CORES)

    nt = n_loc // 128        # 128-pixel tiles (128)
    nt512 = n_loc // 512     # 512-pixel tiles (32)

    # ---- I/O (weights pre-transposed, head-group-sliced, k-tiled on host) --
    # x/r/y are chunk-pair-major [npair*128, 2048]: pair pr rows hold pixels
    # 1024pr..1024pr+1024, col = 1024*(ch%2) + 512k + c.  Each paired DMA is
    # one fully contiguous [128, 2048] block -> 4 KiB packets per partition
    # row (the DMA engines' per-packet overhead dominates below ~4 KiB).
    npair = n_loc // 1024
    x = nc.declare_dram_parameter("x", [npair * 128, 2048], BF16,
                                  isOutput=False)
    r1 = nc.declare_dram_parameter("r1", [npair * 128, 2048], BF16,
                                   isOutput=False)
    r2 = nc.declare_dram_parameter("r2", [npair * 128, 2048], BF16,
                                   isOutput=False)
    # wq: [128, 2*128]  col chunk 128k = Wq.T[128k:128k+128, our 128 channels]
    wq = nc.declare_dram_parameter("wq", [128, 2 * 128], BF16, isOutput=False)
    # wkv_j: [128, 2*256] col chunk 256k = [WkT | WvT](our heads)[128k:, :]
    wkv1 = nc.declare_dram_parameter("wkv1", [128, 2 * 256], BF16, isOutput=False)
    wkv2 = nc.declare_dram_parameter("wkv2", [128, 2 * 256], BF16, isOutput=False)
    # wo: [128, 2*256]  col chunk 256j = Wo.T[(branch j, our heads) rows, :]
    wo = nc.declare_dram_parameter("wo", [128, 2 * 256], BF16, isOutput=False)
    ones4 = nc.declare_dram_parameter("ones4", [128, 32], BF16, isOutput=False)
    ones4T = nc.declare_dram_parameter("ones4T", [128, 128], BF16, isOutput=False)

    y = nc.declare_dram_parameter("y", [npair * 128, 2048], BF16,
                                  isOutput=True)

    refs = [r1, r2]

    with tile.TileContext(nc) as tc:
        with (
            tc.tile_pool(name="weights", bufs=1) as wpool,
            tc.tile_pool(name="persist", bufs=1) as ppool,
            tc.tile_pool(name="io", bufs=6) as iopool,
            tc.tile_pool(name="work", bufs=3) as wkpool,
        ):
            # ---- weights / constants.  Only wkv1 + ones4 are loaded up
            # front; the rest are issued between the first chunk DMAs so the
            # input stream starts as early as possible.
            wkv_t = [wpool.tile([128, 2 * 256], BF16, tag=f"wkv{j}",
                                name=f"wkv_t{j}") for j in range(2)]
            nc.gpsimd.dma_start(wkv_t[0][:], wkv1[:, :])
            ones4_t = wpool.tile([128, 32], BF16, tag="o4")
            nc.sync.dma_start(ones4_t[:], ones4[:, :])
            wq_t = wpool.tile([128, 2 * 128], BF16, tag="wq")
            ones4T_t = wpool.tile([128, 128], BF16, tag="o4T")
            wo_t = wpool.tile([128, 2 * 256], BF16, tag="wo")

            expq = ppool.tile([128, n_loc], BF16, tag="expq")
            nzc = (nt512 + 3) // 4
            zqr = ppool.tile([128, 512 * nzc], BF16, tag="zqr")
            recips = ppool.tile([128, 2], F32, tag="recips")
            compact = ppool.tile([128, 64], BF16, tag="compact")
            wft_sb = ppool.tile([128, 256], BF16, tag="wft_sb")

            CH = 4               # kv: 128-pixel tiles per chunk
            nch = nt // CH       # 32 chunks per branch

            with (
                tc.tile_pool(name="kvstage", bufs=1) as kvpool,
                tc.tile_pool(name="psA", bufs=2, space="PSUM") as psA,
                tc.tile_pool(name="psAcc", bufs=1, space="PSUM") as psAcc,
                tc.tile_pool(name="psQ", bufs=2, space="PSUM") as psQ,
            ):
                ekt_all = kvpool.tile([128, nt * 128], BF16, tag="ekt_all")
                vt_all = kvpool.tile([128, nt * 129], BF16, tag="vt_all")
                vt_v = vt_all.rearrange("p (t c) -> p t c", c=129)
                # ones column per v tile (survives both branches: the per-
                # chunk v copies only touch cols 0:128 of each 129-block)
                nc.vector.memset(vt_v[:, :, 128:129], 1.0)
                # ctx blob: branch j at cols 129j..129j+129; col 129j+128 = zk
                ctx_ps = psAcc.tile([128, 2 * 129], F32, tag="ctx")

                r_hold = [None]

                def pass1(j, ch):
                    if ch % 2 == 0:
                        pr = ch // 2
                        r_hold[0] = iopool.tile(
                            [128, 2048], BF16, tag="rchunk",
                            name=f"r_{j}_{pr}")
                        src = refs[j][128 * pr:128 * (pr + 1), :]
                        if j == 0 and ch == 0:
                            # first pair: 4 quarter DMAs on both queues so
                            # the very first kv matmuls start sooner
                            for qi in range(4):
                                eng = nc.sync if qi % 2 == 0 else nc.gpsimd
                                eng.dma_start(
                                    r_hold[0][:, 512 * qi:512 * (qi + 1)],
                                    src[:, 512 * qi:512 * (qi + 1)])
                        else:
                            dma_eng = nc.sync if pr % 2 == 0 else nc.gpsimd
                            dma_eng.dma_start(r_hold[0][:], src)
                    r_t = r_hold[0]
                    off = 1024 * (ch % 2)
                    kv_ps = psA.tile([128, CH * 256], F32, tag="kv",
                                     name=f"kv_{j}_{ch}")
                    for t in range(CH):
                        for k in range(2):
                            nc.tensor.matmul(
                                kv_ps[:, 256 * t:256 * (t + 1)],
                                r_t[:, off + 512 * k + 128 * t:
                                       off + 512 * k + 128 * (t + 1)],
                                wkv_t[j][:, 256 * k:256 * (k + 1)],
                                start=(k == 0), stop=(k == 1),
                            )
                    ek_sl = ekt_all[:, ch * CH * 128:(ch + 1) * CH * 128]
                    nc.scalar.activation(
                        ek_sl.rearrange("p (t c) -> p t c", t=CH),
                        kv_ps[:].rearrange("p (t c) -> p t c", t=CH)[:, :, 0:128],
                        AF.Exp,
                    )
                    nc.vector.tensor_copy(
                        vt_v[:, ch * CH:(ch + 1) * CH, 0:128],
                        kv_ps[:].rearrange("p (t c) -> p t c", t=CH)[:, :, 128:256],
                    )

                def pass2(j, ch):
                    # ctx accumulation: ek-tile stationary, [v | 1] streamed.
                    # out[c, d] = sum_pix ek[pix, c] v[pix, d]; col 128 = zk.
                    for t in range(ch * CH, (ch + 1) * CH):
                        nc.tensor.matmul(
                            ctx_ps[:, 129 * j:129 * (j + 1)],
                            ekt_all[:, 128 * t:128 * (t + 1)],
                            vt_all[:, 129 * t:129 * (t + 1)],
                            start=(t == 0), stop=(t == nt - 1),
                        )

                x_hold = [None]

                def qchunk(i):
                    base = i * 512
                    if i % 2 == 0:
                        pr = i // 2
                        x_hold[0] = iopool.tile([128, 2048], BF16,
                                                tag="xchunk", name=f"x_{pr}")
                        dma_eng = nc.gpsimd if pr % 2 == 0 else nc.sync
                        dma_eng.dma_start(
                            x_hold[0][:], x[128 * pr:128 * (pr + 1), :])
                    x_t = x_hold[0]
                    off = 1024 * (i % 2)
                    q_ps = psQ.tile([128, 512], F32, tag="q", name=f"q_{i}")
                    for k in range(2):
                        nc.tensor.matmul(
                            q_ps[:], wq_t[:, 128 * k:128 * (k + 1)],
                            x_t[:, off + 512 * k:off + 512 * (k + 1)],
                            start=(k == 0), stop=(k == 1),
                        )
                    nc.scalar.activation(
                        expq[:, base:base + 512], q_ps[:], AF.Exp)

                def zqgroup(tc4):
                    # zq = per-head sums of expq (4 col-tiled concurrent MMs),
                    # reciprocal, then matmul-broadcast back over the 32
                    # partitions of each head and normalize expq in place.
                    zq_ps = psQ.tile([128, 512], F32, tag="q", name=f"zq_{tc4}")
                    for u in range(4):
                        t = 4 * tc4 + u
                        nc.tensor.matmul(
                            zq_ps[32 * u:32 * u + 32, :], ones4_t[:],
                            expq[:, 512 * t:512 * (t + 1)],
                            start=True, stop=True,
                            tile_position=(0, 32 * u),
                        )
                    zq_f = wkpool.tile([128, 512], F32, tag="zq_f",
                                       name=f"zqf_{tc4}")
                    nc.vector.reciprocal_approx_fast(zq_f[:], zq_ps[:])
                    nc.vector.tensor_copy(
                        zqr[:, 512 * tc4:512 * (tc4 + 1)], zq_f[:])
                    for t in range(4 * tc4, 4 * tc4 + 4):
                        u = t % 4
                        zqb_ps = psQ.tile([128, 512], F32, tag="q",
                                          name=f"zqb_{t}")
                        nc.tensor.matmul(
                            zqb_ps[:], ones4T_t[32 * u:32 * u + 4, :],
                            zqr[32 * u:32 * u + 4,
                                512 * tc4:512 * (tc4 + 1)],
                            start=True, stop=True,
                            tile_position=(32 * u, 0),
                        )
                        nc.vector.tensor_mul(
                            expq[:, 512 * t:512 * (t + 1)],
                            expq[:, 512 * t:512 * (t + 1)],
                            zqb_ps[:],
                        )

                # ---- branches; q projection spread across both to even out
                # the DMA load (r + x/2 per branch).  Remaining weight DMAs
                # are dripped in between the chunk DMAs. ----
                for j in range(2):
                    for ch in range(nch):
                        pass1(j, ch)
                        if j == 0 and ch == 0:
                            nc.gpsimd.dma_start(wq_t[:], wq[:, :])
                        if j == 0 and ch == 3:
                            nc.sync.dma_start(ones4T_t[:], ones4T[:, :])
                        if j == 0 and ch == 6:
                            nc.sync.dma_start(wkv_t[1][:], wkv2[:, :])
                        if j == 1 and ch == 2:
                            nc.gpsimd.dma_start(wo_t[:], wo[:, :])
                        if ch > 0:
                            pass2(j, ch - 1)
                        if ch % 2 == 1:
                            qchunk(16 * j + ch // 2)
                        if ch % 8 == 7:
                            zqgroup(4 * j + ch // 8)
                    pass2(j, nch - 1)

                # ---- WF = sum_j WoT_j^T-blocks @ ctx_norm_j, per head -----
                for j in range(2):
                    nc.vector.reciprocal_approx_fast(
                        recips[:, j:j + 1],
                        ctx_ps[:, 129 * j + 128:129 * j + 129])
                for j in range(2):
                    for h in range(4):
                        nc.vector.tensor_scalar_mul(
                            compact[32 * h:32 * (h + 1), 32 * j:32 * j + 32],
                            ctx_ps[32 * h:32 * (h + 1),
                                   129 * j + 32 * h:129 * j + 32 * (h + 1)],
                            recips[32 * h:32 * (h + 1), j:j + 1],
                        )
                wft_ps = psA.tile([128, 256], F32, tag="wft", bufs=1)
                for h in range(4):
                    for j in range(2):
                        nc.tensor.matmul(
                            wft_ps[32 * h:32 * (h + 1), :],
                            compact[32 * h:32 * (h + 1), 32 * j:32 * j + 32],
                            wo_t[32 * h:32 * (h + 1), 256 * j:256 * (j + 1)],
                            start=(j == 0), stop=(j == 1),
                            tile_position=(32 * h, 32 * h),
                        )
                nc.vector.tensor_copy(wft_sb[:], wft_ps[:])

            # ======= Phase C: y = WF @ expq, streamed over pixel tiles ======
            with (
                tc.tile_pool(name="psC", bufs=3, space="PSUM") as psC,
                tc.tile_pool(name="ysb", bufs=5) as ysbpool,
            ):
                y_hold = [None]
                for t in range(nt512):
                    y_ps = psC.tile([128, 1024], F32, tag="y", name=f"y_{t}")
                    for m in range(2):
                        nc.tensor.matmul(
                            y_ps[:, 512 * m:512 * (m + 1)],
                            wft_sb[:, 128 * m:128 * (m + 1)],
                            expq[:, 512 * t:512 * (t + 1)],
                            start=True, stop=True,
                        )
                    if t % 2 == 0:
                        y_hold[0] = ysbpool.tile([128, 2048], BF16, tag="ysb",
                                                 name=f"ysb_{t // 2}")
                    y_sb = y_hold[0]
                    sl = y_sb[:, 1024 * (t % 2):1024 * (t % 2) + 1024]
                    if t % 2 == 0:
                        nc.vector.tensor_copy(sl, y_ps[:])
                    else:
                        nc.scalar.copy(sl, y_ps[:])
                        pr = t // 2
                        dma_eng = nc.sync if pr % 2 == 0 else nc.gpsimd
                        dma_eng.dma_start(
                            y[128 * pr:128 * (pr + 1), :], y_sb[:])

    nc.compile()
    return nc


def _consts():
    ones4 = np.zeros((128, 32), dtype=ml_dtypes.bfloat16)
    for col in range(32):
        a = col % 4
        ones4[32 * a:32 * (a + 1), col] = 1
    ones4T = np.zeros((128, 128), dtype=ml_dtypes.bfloat16)
    for u in range(4):
        for a in range(4):
            ones4T[32 * u + a, 32 * a:32 * (a + 1)] = 1
    return ones4, ones4T


def _ktile(wT):
    """[C_in, C_out] -> [128, (C_in//128)*C_out] k-tiles along the free dim."""
    kin = wT.shape[0] // 128
    return np.concatenate([wT[128 * k:128 * (k + 1), :] for k in range(kin)], axis=1)


def _chunkmajor(arr, n_loc=N):
    """[256, n] -> [npair*128, 2048]: pair pr holds pixels 1024pr..1024pr+1024
    at col = 1024*(ch%2) + 512k + c (k = channel k-tile)."""
    npair = n_loc // 1024
    # [k, p, pr, e, c] -> [pr, p, e, k, c]
    a = arr.reshape(2, 128, npair, 2, 512).transpose(2, 1, 3, 0, 4)
    return a.reshape(npair * 128, 2048)


def _unchunk_y(yarr, n_loc=N):
    """[npair*128, 2048] -> [256, n]: per pair pr, col 1024e + 512m + c of
    partition p is y[128m + p, 512*(2pr + e) + c]."""
    npair = n_loc // 1024
    # [pr, p, e, m, c] -> [m, p, pr, e, c]
    a = yarr.reshape(npair, 128, 2, 2, 512).transpose(3, 1, 0, 2, 4)
    return a.reshape(C, n_loc)


def make_in_maps(x, ref_1, ref_2, Wq, Wk1, Wk2, Wv1, Wv2, Wo, n_loc=N):
    bf = ml_dtypes.bfloat16
    ones4, ones4T = _consts()
    xf = np.asarray(x).reshape(B, C, -1)
    r1f = np.asarray(ref_1).reshape(B, C, -1)
    r2f = np.asarray(ref_2).reshape(B, C, -1)
    WqT, WoT = np.asarray(Wq).T, np.asarray(Wo).T
    WkT = [np.asarray(Wk1).T, np.asarray(Wk2).T]
    WvT = [np.asarray(Wv1).T, np.asarray(Wv2).T]
    gw = {}
    for g in range(2):
        sl = slice(128 * g, 128 * (g + 1))
        wq_g = np.ascontiguousarray(_ktile(WqT[:, sl])).astype(bf)
        wkv_g = [np.ascontiguousarray(
            _ktile(np.concatenate([WkT[j][:, sl], WvT[j][:, sl]], axis=1))
        ).astype(bf) for j in range(2)]
        # Wo rows for our concat channels: branch j block at cols 256j
        wo_g = np.ascontiguousarray(np.concatenate(
            [WoT[128 * g:128 * (g + 1), :],
             WoT[256 + 128 * g:256 + 128 * (g + 1), :]],
            axis=1)).astype(bf)
        gw[g] = (wq_g, wkv_g[0], wkv_g[1], wo_g)
    in_maps = []
    for core in range(N_CORES):
        b, g = core // 2, core % 2
        wq_g, wkv1_g, wkv2_g, wo_g = gw[g]
        in_maps.append({
            "x": np.ascontiguousarray(_chunkmajor(xf[b, :, :n_loc], n_loc)).astype(bf),
            "r1": np.ascontiguousarray(_chunkmajor(r1f[b, :, :n_loc], n_loc)).astype(bf),
            "r2": np.ascontiguousarray(_chunkmajor(r2f[b, :, :n_loc], n_loc)).astype(bf),
            "wq": wq_g, "wkv1": wkv1_g, "wkv2": wkv2_g, "wo": wo_g,
            "ones4": ones4, "ones4T": ones4T,
        })
    return in_maps


_NC_CACHE = {}


def kernel(x, ref_1, ref_2, Wq, Wk1, Wk2, Wv1, Wv2, Wo, _trace=False):
    n_loc = N
    if n_loc not in _NC_CACHE:
        _NC_CACHE[n_loc] = build_nc(n_loc)
    nc = _NC_CACHE[n_loc]
    in_maps = make_in_maps(x, ref_1, ref_2, Wq, Wk1, Wk2, Wv1, Wv2, Wo, n_loc)
    res = run_bass_kernel_spmd(nc, in_maps, core_ids=list(range(N_CORES)),
                               trace=_trace)
    out = np.empty((B, C, n_loc), dtype=np.float32)
    for b in range(B):
        out[b] = (_unchunk_y(res.results[2 * b]["y"].astype(np.float32), n_loc)
                  + _unchunk_y(res.results[2 * b + 1]["y"].astype(np.float32), n_loc))
    if _trace:
        kernel.last_results = res
    return out.reshape(B, C, H, W)
